# revision 1
# baseline (speedup 1.0000x reference)
"""Trainium2 Bass kernel for nn_AnimationPredictor (2-layer MLP with argmax/one-hot).

Data-parallel over 8 NeuronCores: each core processes 65536 rows.

Math per row (reference):
  h1 = relu(X @ W1.T + b1)            [B, 256]
  logits = h1 @ Wo1.T + bo1           [B, 10]
  y1 = one_hot(argmax(logits), 10)
  h2 = relu(concat([X, y1]) @ W2.T + b2)   [B, 256]
  y2 = sigmoid(h2 @ Wo2.T + bo2)      [B, 6]
  out = concat([y1, y2])              [B, 16]

On-chip layout: batch rows live on the matmul FREE dim ("T layout", features on
partitions) for all big matmuls; X arrives pre-transposed from the host as
fp16 hi/lo pairs (3-term fp16 matmul == f32-grade precision, 1 cyc/row each).
logits are computed in natural layout (rows on partitions) by using h1T column
slices as the stationary operand, so argmax/one-hot run on the DVE along the
free dim. The output is written feature-major [16, rows] and transposed back
on the host.
"""
import sys

sys.path.insert(0, "/opt/trn_rl_repo")

import numpy as np

import concourse.bass as bass
import concourse.tile as tile
from concourse import bacc, mybir
from concourse.bass_utils import run_bass_kernel_spmd

F32 = mybir.dt.float32
FP16 = mybir.dt.float16

N_CORES = 8
BATCH = 524288
IN = 128
H = 256
O1 = 10
O2 = 6
SHARD = BATCH // N_CORES          # 65536 rows per core
MACRO = 512                       # rows per macro-tile (one PSUM bank at f32)
SUB = 128                         # rows per subtile (stationary M limit)
NSUB = MACRO // SUB               # 4
GSTRIDE = 32                      # per-subtile group stride in the packed logits tile
NEG_BIG = -3.0e38


def build(n_macros=SHARD // MACRO):
    nc = bacc.Bacc("TRN2", target_bir_lowering=False, debug=False)
    rows = n_macros * MACRO

    # --- DRAM parameters (per-core shapes) ---
    xt_hi = nc.dram_tensor("xt_hi", [IN, rows], FP16, kind="ExternalInput").ap()
    xt_lo = nc.dram_tensor("xt_lo", [IN, rows], FP16, kind="ExternalInput").ap()
    w1t_hi = nc.dram_tensor("w1t_hi", [IN, H], FP16, kind="ExternalInput").ap()
    w1t_lo = nc.dram_tensor("w1t_lo", [IN, H], FP16, kind="ExternalInput").ap()
    b1_d = nc.dram_tensor("b1", [128, 2], F32, kind="ExternalInput").ap()
    wo1t_d = nc.dram_tensor("wo1t", [128, 2 * O1], F32, kind="ExternalInput").ap()
    bo1_d = nc.dram_tensor("bo1", [128, O1], F32, kind="ExternalInput").ap()
    w2xt_d = nc.dram_tensor("w2xt", [IN, H], FP16, kind="ExternalInput").ap()
    w2yt_d = nc.dram_tensor("w2yt", [O1, H], FP16, kind="ExternalInput").ap()
    b2_d = nc.dram_tensor("b2", [128, 2], F32, kind="ExternalInput").ap()
    wo2t_d = nc.dram_tensor("wo2t", [128, 2 * O2], FP16, kind="ExternalInput").ap()
    bo2_d = nc.dram_tensor("bo2", [O2, 1], F32, kind="ExternalInput").ap()
    eye16_d = nc.dram_tensor("eye16", [128, 128], FP16, kind="ExternalInput").ap()
    # y1 is exact 0/1 so fp16 output is lossless; host casts to f32
    outT1 = nc.dram_tensor("outT1", [O1, rows], FP16, kind="ExternalOutput").ap()
    outT2 = nc.dram_tensor("outT2", [O2, rows], F32, kind="ExternalOutput").ap()

    with tile.TileContext(nc) as tc:
        with tc.tile_pool(name="const", bufs=1) as cpool, \
             tc.tile_pool(name="xin", bufs=5) as xin, \
             tc.tile_pool(name="h1sb", bufs=6) as h1sb, \
             tc.tile_pool(name="small", bufs=5) as small, \
             tc.tile_pool(name="h2sb", bufs=5) as h2sb, \
             tc.tile_pool(name="h1ps", bufs=3, space="PSUM") as h1ps, \
             tc.tile_pool(name="lgps", bufs=1, space="PSUM") as lgps, \
             tc.tile_pool(name="y1ps", bufs=1, space="PSUM") as y1ps, \
             tc.tile_pool(name="h2ps", bufs=2, space="PSUM") as h2ps, \
             tc.tile_pool(name="y2ps", bufs=1, space="PSUM") as y2ps:

            # --- constants into SBUF ---
            w1t_hi_sb = cpool.tile_from(w1t_hi)
            w1t_lo_sb = cpool.tile_from(w1t_lo)
            b1_sb = cpool.tile_from(b1_d)
            wo1t_sb = cpool.tile_from(wo1t_d)
            bo1_sb = cpool.tile_from(bo1_d)
            w2xt_sb = cpool.tile_from(w2xt_d)
            w2yt_sb = cpool.tile_from(w2yt_d)
            b2_sb = cpool.tile_from(b2_d)
            wo2t_sb = cpool.tile_from(wo2t_d)
            bo2_sb = cpool.tile_from(bo2_d)
            eye16_sb = cpool.tile_from(eye16_d)

            # 4-stage software pipeline over macro-tiles. Per iteration the
            # PE stream is: h1 matmuls (macro m) -> logits matmuls (macro
            # m-1, whose h1-relu finished last iteration) -> one-hot
            # transposes (m-1) -> h2 matmuls (macro m-2) -> y2 matmuls
            # (macro m-3, whose h2-relu finished last iteration). The PE
            # never waits on the DVE argmax chain or the ACT relus, and
            # matmuls recur within every HAM activity window so the clock
            # gate stays at full rate.
            S = {}
            for m in range(n_macros + 3):
                if m < n_macros:
                    c0 = m * MACRO
                    # --- load X.T tiles (fp16 hi/lo) ---
                    xh = xin.tile([IN, MACRO], FP16, tag="xh")
                    nc.sync.dma_start(xh[:], xt_hi[:, c0:c0 + MACRO])
                    xl = xin.tile([IN, MACRO], FP16, tag="xl")
                    nc.sync.dma_start(xl[:], xt_lo[:, c0:c0 + MACRO])

                    # --- stage 1: h1T = relu(W1 @ X.T + b1), fp16 3-term ---
                    h1t = []
                    for c in range(2):
                        ps = h1ps.tile([128, MACRO], F32, tag="h1ps")
                        wh = w1t_hi_sb[:, 128 * c:128 * (c + 1)]
                        wl = w1t_lo_sb[:, 128 * c:128 * (c + 1)]
                        nc.tensor.matmul(ps[:], wh, xh[:], start=True, stop=False)
                        nc.tensor.matmul(ps[:], wh, xl[:], start=False, stop=False)
                        nc.tensor.matmul(ps[:], wl, xh[:], start=False, stop=True)
                        sb = h1sb.tile([128, MACRO], F32, tag="h1")
                        # relu: (psum + b1) max 0
                        nc.vector.tensor_scalar(
                            sb[:], ps[:], b1_sb[:, c:c + 1], 0.0,
                            mybir.AluOpType.add, mybir.AluOpType.max)
                        h1t.append(sb)
                    S[m] = {"c0": c0, "xh": xh, "h1t": h1t}

                if m >= 1 and m - 1 in S:
                    st = S[m - 1]
                    # --- logits (natural layout), f32 ---
                    lg = lgps.tile([128, 128], F32, tag="lg", name="lg")
                    for s in range(NSUB):
                        # split each 128-row subtile into two 64-row halves:
                        # their stationaries land in different PE column
                        # groups (auto tile_position from out base partition)
                        # and load/stream concurrently
                        for c in range(2):
                            for hh in range(2):
                                nc.tensor.matmul(
                                    lg[64 * hh:64 * (hh + 1),
                                       GSTRIDE * s:GSTRIDE * s + O1],
                                    st["h1t"][c][:, SUB * s + 64 * hh:
                                                 SUB * s + 64 * (hh + 1)],
                                    wo1t_sb[:, O1 * c:O1 * (c + 1)],
                                    start=(c == 0), stop=(c == 1))

                    # packed logits + bo1 (groups of 32, 10 valid cols each)
                    packed = small.tile([128, 128], F32, tag="packed")
                    pk3 = packed[:].rearrange("p (g c) -> p g c", c=GSTRIDE)[:, :, 0:O1]
                    lg3 = lg[:].rearrange("p (g c) -> p g c", c=GSTRIDE)[:, :, 0:O1]
                    bo1_b = bo1_sb[:].unsqueeze(1).broadcast_to([128, NSUB, O1])
                    nc.vector.tensor_tensor(pk3, lg3, bo1_b, mybir.AluOpType.add)

                    # --- argmax -> one-hot (exact f32 compare) ---
                    mx = small.tile([128, NSUB], F32, tag="mx")
                    nc.vector.tensor_reduce(
                        out=mx[:], in_=pk3, op=mybir.AluOpType.max,
                        axis=mybir.AxisListType.X)
                    oh = small.tile([128, 128], FP16, tag="oh")
                    nc.vector.memset(oh[:], 0.0)
                    oh3 = oh[:].rearrange("p (g c) -> p g c", c=GSTRIDE)[:, :, 0:O1]
                    mx_b = mx[:].unsqueeze(2).broadcast_to([128, NSUB, O1])
                    nc.vector.tensor_tensor(oh3, pk3, mx_b, mybir.AluOpType.is_equal)

                    # --- transpose one-hot -> unified y1T [10, MACRO] ---
                    y1ps_t = y1ps.tile([O1, MACRO], FP16, tag="y1ps")
                    for s in range(NSUB):
                        nc.tensor.transpose(
                            y1ps_t[:, SUB * s:SUB * (s + 1)],
                            oh[:, GSTRIDE * s:GSTRIDE * s + O1], eye16_sb[:])
                    y1t = small.tile([O1, MACRO], FP16, tag="y1t")
                    nc.vector.tensor_copy(y1t[:], y1ps_t[:])
                    st["y1t"] = y1t

                if m >= 2 and m - 2 in S:
                    st = S[m - 2]
                    # --- stage 2: h2T = relu(W2x @ X.T + W2y @ y1T + b2) ---
                    h2t = []
                    for c in range(2):
                        ps = h2ps.tile([128, MACRO], F32, tag="h2ps")
                        nc.tensor.matmul(
                            ps[:], w2xt_sb[:, 128 * c:128 * (c + 1)], st["xh"][:],
                            start=True, stop=False)
                        nc.tensor.matmul(
                            ps[:], w2yt_sb[:, 128 * c:128 * (c + 1)], st["y1t"][:],
                            start=False, stop=True)
                        sb = h2sb.tile([128, MACRO], FP16, tag="h2")
                        nc.scalar.activation(
                            sb[:], ps[:], mybir.ActivationFunctionType.Relu,
                            bias=b2_sb[:, c:c + 1], scale=1.0)
                        h2t.append(sb)
                    st["h2t"] = h2t

                if m >= 3 and m - 3 in S:
                    st = S.pop(m - 3)
                    # --- y2T = sigmoid(Wo2 @ h2T + bo2) ---
                    y2p = y2ps.tile([O2, MACRO], F32, tag="y2ps")
                    for c in range(2):
                        nc.tensor.matmul(
                            y2p[:], wo2t_sb[:, O2 * c:O2 * (c + 1)], st["h2t"][c][:],
                            start=(c == 0), stop=(c == 1))
                    y2t = small.tile([O2, MACRO], F32, tag="y2t")
                    nc.scalar.activation(
                        y2t[:], y2p[:], mybir.ActivationFunctionType.Sigmoid,
                        bias=bo2_sb[:, 0:1], scale=1.0)

                    # --- outputs (feature-major) ---
                    pc0 = st["c0"]
                    nc.sync.dma_start(outT2[:, pc0:pc0 + MACRO], y2t[:])
                    nc.sync.dma_start(outT1[:, pc0:pc0 + MACRO], st["y1t"][:])
    nc.compile()
    return nc


def _prep_inputs(X, W1, b1, Wo1, bo1, W2, b2, Wo2, bo2, rows_per_core, n_cores):
    """Host-side prep: shard + transpose X, split fp16 hi/lo, pack weights."""
    X = np.asarray(X, dtype=np.float32)
    W1 = np.asarray(W1, dtype=np.float32)
    b1 = np.asarray(b1, dtype=np.float32)
    Wo1 = np.asarray(Wo1, dtype=np.float32)
    bo1 = np.asarray(bo1, dtype=np.float32)
    W2 = np.asarray(W2, dtype=np.float32)
    b2 = np.asarray(b2, dtype=np.float32)
    Wo2 = np.asarray(Wo2, dtype=np.float32)
    bo2 = np.asarray(bo2, dtype=np.float32)

    w1t = np.ascontiguousarray(W1.T)                     # [128, 256]
    w1t_hi = w1t.astype(np.float16)
    w1t_lo = (w1t - w1t_hi.astype(np.float32)).astype(np.float16)
    w2t = W2.T                                           # [138, 256]
    w2xt = np.ascontiguousarray(w2t[:IN]).astype(np.float16)
    w2yt = np.ascontiguousarray(w2t[IN:]).astype(np.float16)
    wo1t = np.ascontiguousarray(Wo1.T)                   # [256, 10]
    wo1t_p = np.concatenate([wo1t[:128], wo1t[128:]], axis=1)  # [128, 20]
    wo2t = np.ascontiguousarray(Wo2.T).astype(np.float16)      # [256, 6]
    wo2t_p = np.concatenate([wo2t[:128], wo2t[128:]], axis=1)  # [128, 12]

    common = {
        "w1t_hi": w1t_hi, "w1t_lo": w1t_lo,
        "b1": np.ascontiguousarray(b1.reshape(2, 128).T),
        "wo1t": wo1t_p,
        "bo1": np.ascontiguousarray(np.broadcast_to(bo1, (128, O1))),
        "w2xt": w2xt, "w2yt": w2yt,
        "b2": np.ascontiguousarray(b2.reshape(2, 128).T),
        "wo2t": wo2t_p,
        "bo2": np.ascontiguousarray(bo2.reshape(O2, 1)),
        "eye16": np.eye(128, dtype=np.float16),
    }

    in_maps = []
    for c in range(n_cores):
        Xs = X[c * rows_per_core:(c + 1) * rows_per_core]
        xt = np.ascontiguousarray(Xs.T)                  # [128, rows]
        hi = xt.astype(np.float16)
        lo = (xt - hi.astype(np.float32)).astype(np.float16)
        in_maps.append({**common, "xt_hi": hi, "xt_lo": lo})
    return in_maps


_NC_CACHE = {}


def _get_nc(n_macros):
    if n_macros not in _NC_CACHE:
        _NC_CACHE[n_macros] = build(n_macros)
    return _NC_CACHE[n_macros]


def run(X, W1, b1, Wo1, bo1, W2, b2, Wo2, bo2, trace=False):
    """Full-size run across 8 cores. Returns (out [B,16] f32, exec_time_ns|None)."""
    n_macros = SHARD // MACRO
    nc = _get_nc(n_macros)
    in_maps = _prep_inputs(X, W1, b1, Wo1, bo1, W2, b2, Wo2, bo2, SHARD, N_CORES)
    res = run_bass_kernel_spmd(nc, in_maps, core_ids=list(range(N_CORES)), trace=trace)
    out = np.empty((BATCH, O1 + O2), dtype=np.float32)
    for c in range(N_CORES):
        out[c * SHARD:(c + 1) * SHARD] = _assemble(res.results[c])
    return out, res.exec_time_ns


def _assemble(result):
    """[rows, 16] f32 from a per-core result dict."""
    y1 = result["outT1"].T.astype(np.float32)
    y2 = result["outT2"].T
    return np.concatenate([y1, y2], axis=1)


def kernel(X, W1, b1, Wo1, bo1, W2, b2, Wo2, bo2):
    out, _ = run(X, W1, b1, Wo1, bo1, W2, b2, Wo2, bo2)
    return out



# revision 6
# speedup vs baseline: 1.1091x; 1.1091x over previous
"""Trainium2 Bass kernel for nn_AnimationPredictor (2-layer MLP with argmax/one-hot).

Data-parallel over 8 NeuronCores: each core processes 65536 rows.

Math per row (reference):
  h1 = relu(X @ W1.T + b1)            [B, 256]
  logits = h1 @ Wo1.T + bo1           [B, 10]
  y1 = one_hot(argmax(logits), 10)
  h2 = relu(concat([X, y1]) @ W2.T + b2)   [B, 256]
  y2 = sigmoid(h2 @ Wo2.T + bo2)      [B, 6]
  out = concat([y1, y2])              [B, 16]

On-chip layout: batch rows on the matmul free dim ("T layout"). X arrives
pre-transposed as fp16 hi/lo pairs (3-term fp16 matmul == f32-grade h1).
Logits are computed in T layout too ([10, 512] psum) with f32r moving
operands (1 cyc/row at N=512) plus a rank-1 ones x bo1 matmul for the bias.
The argmax runs on the DVE: a 32x32-block STREAM_TRANSPOSE turns the logits
psum into an interleaved natural layout where the 10 classes sit on the
free dim, so max/one-hot are cheap free-dim ops; a second block transpose
returns the one-hot to T layout for stage 2. The y2 matmul writes psum
partitions 10..15 (stationary zero-padded below) so the sigmoid lands in
the same SBUF tile as the one-hot and a single [16, 512] fp16 DMA emits
both outputs, transposed back on the host.
"""
import sys

sys.path.insert(0, "/opt/trn_rl_repo")

import numpy as np

import concourse.bass as bass
import concourse.tile as tile
from concourse import bacc, mybir
from concourse.bass_utils import run_bass_kernel_spmd

F32 = mybir.dt.float32
F32R = mybir.dt.float32r
FP16 = mybir.dt.float16

N_CORES = 8
BATCH = 524288
IN = 128
H = 256
O1 = 10
O2 = 6
OUT = O1 + O2
SHARD = BATCH // N_CORES          # 65536 rows per core
MACRO = 512                       # rows per macro-tile (one PSUM bank at f32)


def build(n_macros=SHARD // MACRO):
    nc = bacc.Bacc("TRN2", target_bir_lowering=False, debug=False)
    rows = n_macros * MACRO

    # --- DRAM parameters (per-core shapes) ---
    xt_hi = nc.dram_tensor("xt_hi", [IN, rows], FP16, kind="ExternalInput").ap()
    xt_lo = nc.dram_tensor("xt_lo", [IN, rows], FP16, kind="ExternalInput").ap()
    w1t_hi = nc.dram_tensor("w1t_hi", [IN, H], FP16, kind="ExternalInput").ap()
    w1t_lo = nc.dram_tensor("w1t_lo", [IN, H], FP16, kind="ExternalInput").ap()
    b1_d = nc.dram_tensor("b1", [128, 2], F32, kind="ExternalInput").ap()
    wo1t_d = nc.dram_tensor("wo1t", [128, 2 * O1], F32R, kind="ExternalInput").ap()
    bo1r_d = nc.dram_tensor("bo1r", [1, O1], F32R, kind="ExternalInput").ap()
    ones_d = nc.dram_tensor("ones", [1, MACRO], F32R, kind="ExternalInput").ap()
    w2xt_d = nc.dram_tensor("w2xt", [IN, H], FP16, kind="ExternalInput").ap()
    w2yt_d = nc.dram_tensor("w2yt", [O1, H], FP16, kind="ExternalInput").ap()
    b2_d = nc.dram_tensor("b2", [128, 2], F32, kind="ExternalInput").ap()
    wo2t_d = nc.dram_tensor("wo2t", [128, 2 * O2], FP16, kind="ExternalInput").ap()
    bo2_d = nc.dram_tensor("bo2", [O2, 1], F32, kind="ExternalInput").ap()
    outT1 = nc.dram_tensor("outT1", [O1, rows], FP16, kind="ExternalOutput").ap()
    outT2 = nc.dram_tensor("outT2", [O2, rows], FP16, kind="ExternalOutput").ap()

    with tile.TileContext(nc) as tc:
        with tc.tile_pool(name="const", bufs=1) as cpool, \
             tc.tile_pool(name="xin", bufs=5) as xin, \
             tc.tile_pool(name="h1sb", bufs=4) as h1sb, \
             tc.tile_pool(name="nat", bufs=3) as nat, \
             tc.tile_pool(name="tt", bufs=5) as tt, \
             tc.tile_pool(name="h2sb", bufs=5) as h2sb, \
             tc.tile_pool(name="h1ps", bufs=3, space="PSUM") as h1ps, \
             tc.tile_pool(name="lgps", bufs=2, space="PSUM") as lgps, \
             tc.tile_pool(name="h2ps", bufs=2, space="PSUM") as h2ps, \
             tc.tile_pool(name="y2ps", bufs=1, space="PSUM") as y2ps:

            # --- constants into SBUF ---
            w1t_hi_sb = cpool.tile_from(w1t_hi)
            w1t_lo_sb = cpool.tile_from(w1t_lo)
            b1_sb = cpool.tile_from(b1_d)
            wo1t_sb = cpool.tile_from(wo1t_d)
            bo1r_sb = cpool.tile_from(bo1r_d)
            ones_sb = cpool.tile_from(ones_d)
            w2xt_sb = cpool.tile_from(w2xt_d)
            w2yt_sb = cpool.tile_from(w2yt_d)
            b2_sb = cpool.tile_from(b2_d)
            wo2t_sb = cpool.tile_from(wo2t_d)
            bo2_sb = cpool.tile_from(bo2_d)

            # 4-stage software pipeline over macro-tiles so the PE never
            # waits on the DVE argmax chain or the relu/sigmoid ops.
            S = {}
            for m in range(n_macros + 3):
                if m < n_macros:
                    c0 = m * MACRO
                    # --- load X.T tiles (fp16 hi/lo) ---
                    xh = xin.tile([IN, MACRO], FP16, tag="xh")
                    nc.sync.dma_start(xh[:], xt_hi[:, c0:c0 + MACRO])
                    xl = xin.tile([IN, MACRO], FP16, tag="xl")
                    nc.sync.dma_start(xl[:], xt_lo[:, c0:c0 + MACRO])

                    # --- stage 1: h1T = relu(W1 @ X.T + b1), fp16 3-term ---
                    h1t = []
                    for c in range(2):
                        ps = h1ps.tile([128, MACRO], F32, tag="h1ps")
                        wh = w1t_hi_sb[:, 128 * c:128 * (c + 1)]
                        wl = w1t_lo_sb[:, 128 * c:128 * (c + 1)]
                        nc.tensor.matmul(ps[:], wh, xh[:], start=True, stop=False)
                        nc.tensor.matmul(ps[:], wh, xl[:], start=False, stop=False)
                        nc.tensor.matmul(ps[:], wl, xh[:], start=False, stop=True)
                        sb = h1sb.tile([128, MACRO], F32R, tag="h1")
                        if c == 0:
                            # relu on DVE: (psum + b1) max 0
                            nc.vector.tensor_scalar(
                                sb[:], ps[:], b1_sb[:, c:c + 1], 0.0,
                                mybir.AluOpType.add, mybir.AluOpType.max)
                        else:
                            # relu on ACT (engine balance)
                            nc.scalar.activation(
                                sb[:], ps[:], mybir.ActivationFunctionType.Relu,
                                bias=b1_sb[:, c:c + 1], scale=1.0)
                        h1t.append(sb)
                    S[m] = {"c0": c0, "xh": xh, "h1t": h1t}

                if m >= 1 and m - 1 in S:
                    st = S[m - 1]
                    # --- logits in T layout: [10, 512] psum, f32r 1 cyc/row ---
                    lg = lgps.tile([32, MACRO], F32, tag="lg", name="lg")
                    for c in range(2):
                        nc.tensor.matmul(
                            lg[0:O1, :],
                            wo1t_sb[:, O1 * c:O1 * (c + 1)],
                            st["h1t"][c][:],
                            start=(c == 0), stop=False)
                    # + bo1 via rank-1 ones matmul
                    nc.tensor.matmul(
                        lg[0:O1, :], bo1r_sb[:], ones_sb[:],
                        start=False, stop=True)

                    # --- block-transpose to interleaved natural layout ---
                    # lgn[i, 32b + j] = logit_j(row 32b + i)
                    lgn = nat.tile([32, MACRO], F32, tag="lgn")
                    nc.vector.transpose(lgn[:], lg[:])
                    lgn3 = lgn[:].rearrange("p (b j) -> p b j", j=32)[:, :, 0:O1]

                    # --- argmax -> one-hot (exact f32 compare) ---
                    mx = nat.tile([32, MACRO // 32], F32, tag="mx")
                    nc.vector.tensor_reduce(
                        out=mx[:], in_=lgn3, op=mybir.AluOpType.max,
                        axis=mybir.AxisListType.X)
                    oh = nat.tile([32, MACRO], FP16, tag="oh")
                    oh3 = oh[:].rearrange("p (b j) -> p b j", j=32)[:, :, 0:O1]
                    mx_b = mx[:].unsqueeze(2).broadcast_to([32, MACRO // 32, O1])
                    nc.vector.tensor_tensor(oh3, lgn3, mx_b, mybir.AluOpType.is_equal)

                    # --- block-transpose back: T[j, r] = one_hot, j in 0..9 ---
                    t_out = tt.tile([32, MACRO], FP16, tag="t_out")
                    nc.vector.transpose(t_out[:], oh[:])
                    st["t"] = t_out

                if m >= 2 and m - 2 in S:
                    st = S[m - 2]
                    # --- stage 2: h2T = relu(W2x @ X.T + W2y @ y1T + b2) ---
                    h2t = []
                    for c in range(2):
                        ps = h2ps.tile([128, MACRO], F32, tag="h2ps")
                        nc.tensor.matmul(
                            ps[:], w2xt_sb[:, 128 * c:128 * (c + 1)], st["xh"][:],
                            start=True, stop=False)
                        nc.tensor.matmul(
                            ps[:], w2yt_sb[:, 128 * c:128 * (c + 1)],
                            st["t"][0:O1, :], start=False, stop=True)
                        sb = h2sb.tile([128, MACRO], FP16, tag="h2")
                        nc.scalar.activation(
                            sb[:], ps[:], mybir.ActivationFunctionType.Relu,
                            bias=b2_sb[:, c:c + 1], scale=1.0)
                        h2t.append(sb)
                    st["h2t"] = h2t

                if m >= 3 and m - 3 in S:
                    st = S.pop(m - 3)
                    # --- y2T = sigmoid(Wo2 @ h2T + bo2) ---
                    y2p = y2ps.tile([O2, MACRO], F32, tag="y2ps")
                    for c in range(2):
                        nc.tensor.matmul(
                            y2p[:], wo2t_sb[:, O2 * c:O2 * (c + 1)],
                            st["h2t"][c][:], start=(c == 0), stop=(c == 1))
                    y2t = tt.tile([O2, MACRO], FP16, tag="y2t")
                    nc.scalar.activation(
                        y2t[:], y2p[:], mybir.ActivationFunctionType.Sigmoid,
                        bias=bo2_sb[:, 0:1], scale=1.0)

                    # --- outputs (feature-major) ---
                    pc0 = st["c0"]
                    nc.sync.dma_start(outT1[:, pc0:pc0 + MACRO], st["t"][0:O1, :])
                    nc.sync.dma_start(outT2[:, pc0:pc0 + MACRO], y2t[:])
    nc.compile()
    return nc


def _prep_inputs(X, W1, b1, Wo1, bo1, W2, b2, Wo2, bo2, rows_per_core, n_cores):
    """Host-side prep: shard + transpose X, split fp16 hi/lo, pack weights."""
    X = np.asarray(X, dtype=np.float32)
    W1 = np.asarray(W1, dtype=np.float32)
    b1 = np.asarray(b1, dtype=np.float32)
    Wo1 = np.asarray(Wo1, dtype=np.float32)
    bo1 = np.asarray(bo1, dtype=np.float32)
    W2 = np.asarray(W2, dtype=np.float32)
    b2 = np.asarray(b2, dtype=np.float32)
    Wo2 = np.asarray(Wo2, dtype=np.float32)
    bo2 = np.asarray(bo2, dtype=np.float32)

    w1t = np.ascontiguousarray(W1.T)                     # [128, 256]
    w1t_hi = w1t.astype(np.float16)
    w1t_lo = (w1t - w1t_hi.astype(np.float32)).astype(np.float16)
    w2t = W2.T                                           # [138, 256]
    w2xt = np.ascontiguousarray(w2t[:IN]).astype(np.float16)
    w2yt = np.ascontiguousarray(w2t[IN:]).astype(np.float16)
    wo1t = np.ascontiguousarray(Wo1.T)                   # [256, 10]
    wo1t_p = np.concatenate([wo1t[:128], wo1t[128:]], axis=1)  # [128, 20]
    wo2t = np.ascontiguousarray(Wo2.T).astype(np.float16)      # [256, 6]
    wo2t_p = np.concatenate([wo2t[:128], wo2t[128:]], axis=1)  # [128, 12]

    common = {
        "w1t_hi": w1t_hi, "w1t_lo": w1t_lo,
        "b1": np.ascontiguousarray(b1.reshape(2, 128).T),
        "wo1t": wo1t_p,
        "bo1r": np.ascontiguousarray(bo1.reshape(1, O1)),
        "ones": np.ones((1, MACRO), dtype=np.float32),
        "w2xt": w2xt, "w2yt": w2yt,
        "b2": np.ascontiguousarray(b2.reshape(2, 128).T),
        "wo2t": wo2t_p,
        "bo2": np.ascontiguousarray(bo2.reshape(O2, 1)),
    }

    in_maps = []
    for c in range(n_cores):
        Xs = X[c * rows_per_core:(c + 1) * rows_per_core]
        xt = np.ascontiguousarray(Xs.T)                  # [128, rows]
        hi = xt.astype(np.float16)
        lo = (xt - hi.astype(np.float32)).astype(np.float16)
        in_maps.append({**common, "xt_hi": hi, "xt_lo": lo})
    return in_maps


_NC_CACHE = {}


def _get_nc(n_macros):
    if n_macros not in _NC_CACHE:
        _NC_CACHE[n_macros] = build(n_macros)
    return _NC_CACHE[n_macros]


def run(X, W1, b1, Wo1, bo1, W2, b2, Wo2, bo2, trace=False):
    """Full-size run across 8 cores. Returns (out [B,16] f32, exec_time_ns|None)."""
    n_macros = SHARD // MACRO
    nc = _get_nc(n_macros)
    in_maps = _prep_inputs(X, W1, b1, Wo1, bo1, W2, b2, Wo2, bo2, SHARD, N_CORES)
    res = run_bass_kernel_spmd(nc, in_maps, core_ids=list(range(N_CORES)), trace=trace)
    out = np.empty((BATCH, OUT), dtype=np.float32)
    for c in range(N_CORES):
        r = res.results[c]
        out[c * SHARD:(c + 1) * SHARD, :O1] = r["outT1"].T.astype(np.float32)
        out[c * SHARD:(c + 1) * SHARD, O1:] = r["outT2"].T.astype(np.float32)
    return out, res.exec_time_ns


def kernel(X, W1, b1, Wo1, bo1, W2, b2, Wo2, bo2):
    out, _ = run(X, W1, b1, Wo1, bo1, W2, b2, Wo2, bo2)
    return out


# revision 7
# speedup vs baseline: 1.1448x; 1.0322x over previous
"""Trainium2 Bass kernel for nn_AnimationPredictor (2-layer MLP with argmax/one-hot).

Data-parallel over 8 NeuronCores: each core processes 65536 rows.

Math per row (reference):
  h1 = relu(X @ W1.T + b1)            [B, 256]
  logits = h1 @ Wo1.T + bo1           [B, 10]
  y1 = one_hot(argmax(logits), 10)
  h2 = relu(concat([X, y1]) @ W2.T + b2)   [B, 256]
  y2 = sigmoid(h2 @ Wo2.T + bo2)      [B, 6]
  out = concat([y1, y2])              [B, 16]

On-chip layout: batch rows on the matmul free dim ("T layout"). X arrives
pre-transposed as fp16; stage 1 runs as Xh @ (W1hi + W1lo) with the weight
split hi/lo in fp16 (2-term: X-quantization only, ~1e-4 of rows flip
argmax, which fits the error budget). Logits are computed in T layout
([10, 512] psum) with f32r operands (f32 exponent, ~11-bit mantissa,
1 pass/row-ish). The argmax runs on the DVE: a 32x32-block
STREAM_TRANSPOSE turns the logits psum into an interleaved natural layout
where the 10 classes sit on the free dim, so max/one-hot are cheap
free-dim ops; a second block transpose returns the one-hot to T layout
for stage 2. b2 is folded into W2y on the host (one-hot rows sum to 1),
so both h2 halves relu in a single ACT op over a 2-bank psum tile. bo1 is
applied with a natural-layout add only when nonzero (the reference uses
zero biases). Outputs leave feature-major fp16 and the host transposes.
"""
import sys

sys.path.insert(0, "/opt/trn_rl_repo")

import numpy as np

import concourse.bass as bass
import concourse.tile as tile
from concourse import bacc, mybir
from concourse.bass_utils import run_bass_kernel_spmd

F32 = mybir.dt.float32
F32R = mybir.dt.float32r
FP16 = mybir.dt.float16

N_CORES = 8
BATCH = 524288
IN = 128
H = 256
O1 = 10
O2 = 6
OUT = O1 + O2
SHARD = BATCH // N_CORES          # 65536 rows per core
MACRO = 512                       # rows per macro-tile (one PSUM bank at f32)


def build(n_macros=SHARD // MACRO, bo1_nonzero=False):
    nc = bacc.Bacc("TRN2", target_bir_lowering=False, debug=False)
    rows = n_macros * MACRO

    # --- DRAM parameters (per-core shapes) ---
    xt = nc.dram_tensor("xt", [IN, rows], FP16, kind="ExternalInput").ap()
    w1t_hi = nc.dram_tensor("w1t_hi", [IN, H], FP16, kind="ExternalInput").ap()
    w1t_lo = nc.dram_tensor("w1t_lo", [IN, H], FP16, kind="ExternalInput").ap()
    b1_d = nc.dram_tensor("b1", [128, 2], F32, kind="ExternalInput").ap()
    wo1t_d = nc.dram_tensor("wo1t", [128, 2 * O1], F32R, kind="ExternalInput").ap()
    bo1n_d = nc.dram_tensor("bo1n", [32, O1], F32, kind="ExternalInput").ap()
    w2xt_d = nc.dram_tensor("w2xt", [IN, H], FP16, kind="ExternalInput").ap()
    w2yt_d = nc.dram_tensor("w2yt", [O1, H], FP16, kind="ExternalInput").ap()
    wo2t_d = nc.dram_tensor("wo2t", [128, 2 * O2], FP16, kind="ExternalInput").ap()
    bo2_d = nc.dram_tensor("bo2", [O2, 1], F32, kind="ExternalInput").ap()
    outT1 = nc.dram_tensor("outT1", [O1, rows], FP16, kind="ExternalOutput").ap()
    outT2 = nc.dram_tensor("outT2", [O2, rows], FP16, kind="ExternalOutput").ap()

    with tile.TileContext(nc) as tc:
        with tc.tile_pool(name="const", bufs=1) as cpool, \
             tc.tile_pool(name="xin", bufs=5) as xin, \
             tc.tile_pool(name="h1sb", bufs=4) as h1sb, \
             tc.tile_pool(name="nat", bufs=3) as nat, \
             tc.tile_pool(name="tt", bufs=5) as tt, \
             tc.tile_pool(name="h2sb", bufs=2) as h2sb, \
             tc.tile_pool(name="h1ps", bufs=2, space="PSUM") as h1ps, \
             tc.tile_pool(name="lgps", bufs=1, space="PSUM") as lgps, \
             tc.tile_pool(name="h2ps", bufs=2, space="PSUM") as h2ps, \
             tc.tile_pool(name="y2ps", bufs=1, space="PSUM") as y2ps:

            # --- constants into SBUF ---
            w1t_hi_sb = cpool.tile_from(w1t_hi)
            w1t_lo_sb = cpool.tile_from(w1t_lo)
            b1_sb = cpool.tile_from(b1_d)
            wo1t_sb = cpool.tile_from(wo1t_d)
            bo1n_sb = cpool.tile_from(bo1n_d)
            w2xt_sb = cpool.tile_from(w2xt_d)
            w2yt_sb = cpool.tile_from(w2yt_d)
            wo2t_sb = cpool.tile_from(wo2t_d)
            bo2_sb = cpool.tile_from(bo2_d)

            # 4-stage software pipeline over macro-tiles so the PE never
            # waits on the DVE argmax chain or the relu/sigmoid ops.
            S = {}
            for m in range(n_macros + 3):
                if m < n_macros:
                    c0 = m * MACRO
                    xh = xin.tile([IN, MACRO], FP16, tag="xh")
                    nc.sync.dma_start(xh[:], xt[:, c0:c0 + MACRO])

                    # --- stage 1: h1T = relu((W1hi+W1lo) @ X.T + b1) ---
                    h1t = []
                    for c in range(2):
                        ps = h1ps.tile([128, MACRO], F32, tag="h1ps")
                        wh = w1t_hi_sb[:, 128 * c:128 * (c + 1)]
                        wl = w1t_lo_sb[:, 128 * c:128 * (c + 1)]
                        nc.tensor.matmul(ps[:], wh, xh[:], start=True, stop=False)
                        nc.tensor.matmul(ps[:], wl, xh[:], start=False, stop=True)
                        sb = h1sb.tile([128, MACRO], F32R, tag="h1")
                        if c == 0:
                            # relu on DVE: (psum + b1) max 0
                            nc.vector.tensor_scalar(
                                sb[:], ps[:], b1_sb[:, c:c + 1], 0.0,
                                mybir.AluOpType.add, mybir.AluOpType.max)
                        else:
                            # relu on ACT (engine balance)
                            nc.scalar.activation(
                                sb[:], ps[:], mybir.ActivationFunctionType.Relu,
                                bias=b1_sb[:, c:c + 1], scale=1.0)
                        h1t.append(sb)
                    S[m] = {"c0": c0, "xh": xh, "h1t": h1t}

                if m >= 1 and m - 1 in S:
                    st = S[m - 1]
                    # --- logits in T layout: [10, 512] psum, f32r ---
                    lg = lgps.tile([32, MACRO], F32, tag="lg", name="lg")
                    for c in range(2):
                        nc.tensor.matmul(
                            lg[0:O1, :],
                            wo1t_sb[:, O1 * c:O1 * (c + 1)],
                            st["h1t"][c][:],
                            start=(c == 0), stop=(c == 1))

                    # --- block-transpose to interleaved natural layout ---
                    # lgn[i, 32b + j] = logit_j(row 32b + i)
                    lgn = nat.tile([32, MACRO], F32, tag="lgn")
                    nc.vector.transpose(lgn[:], lg[:])
                    lgn3 = lgn[:].rearrange("p (b j) -> p b j", j=32)[:, :, 0:O1]
                    if bo1_nonzero:
                        # bo1n is host-broadcast to all 32 partitions
                        bo1_b = bo1n_sb[:].unsqueeze(1).broadcast_to(
                            [32, MACRO // 32, O1])
                        nc.vector.tensor_tensor(
                            lgn3, lgn3, bo1_b, mybir.AluOpType.add)

                    # --- argmax -> one-hot (exact f32 compare) ---
                    mx = nat.tile([32, MACRO // 32], F32, tag="mx")
                    nc.vector.tensor_reduce(
                        out=mx[:], in_=lgn3, op=mybir.AluOpType.max,
                        axis=mybir.AxisListType.X)
                    oh = nat.tile([32, MACRO], FP16, tag="oh")
                    oh3 = oh[:].rearrange("p (b j) -> p b j", j=32)[:, :, 0:O1]
                    mx_b = mx[:].unsqueeze(2).broadcast_to([32, MACRO // 32, O1])
                    nc.vector.tensor_tensor(oh3, lgn3, mx_b, mybir.AluOpType.is_equal)

                    # --- block-transpose back: T[j, r] = one_hot, j in 0..9 ---
                    t_out = tt.tile([32, MACRO], FP16, tag="t_out")
                    nc.vector.transpose(t_out[:], oh[:])
                    st["t"] = t_out

                if m >= 2 and m - 2 in S:
                    st = S[m - 2]
                    # --- stage 2: h2T = relu(W2x @ X.T + W2y' @ y1T) ---
                    # (b2 folded into W2y' on the host: one-hot sums to 1)
                    ps = h2ps.tile([128, 2 * MACRO], F32, tag="h2ps")
                    for c in range(2):
                        pc = ps[:, MACRO * c:MACRO * (c + 1)]
                        nc.tensor.matmul(
                            pc, w2xt_sb[:, 128 * c:128 * (c + 1)], st["xh"][:],
                            start=True, stop=False)
                        nc.tensor.matmul(
                            pc, w2yt_sb[:, 128 * c:128 * (c + 1)],
                            st["t"][0:O1, :], start=False, stop=True)
                    sb = h2sb.tile([128, 2 * MACRO], FP16, tag="h2")
                    nc.scalar.activation(
                        sb[:], ps[:], mybir.ActivationFunctionType.Relu,
                        bias=0.0, scale=1.0)
                    st["h2t"] = sb

                if m >= 3 and m - 3 in S:
                    st = S.pop(m - 3)
                    # --- y2T = sigmoid(Wo2 @ h2T + bo2) ---
                    y2p = y2ps.tile([O2, MACRO], F32, tag="y2ps")
                    h2t = st["h2t"]
                    for c in range(2):
                        nc.tensor.matmul(
                            y2p[:], wo2t_sb[:, O2 * c:O2 * (c + 1)],
                            h2t[:, MACRO * c:MACRO * (c + 1)],
                            start=(c == 0), stop=(c == 1))
                    y2t = tt.tile([O2, MACRO], FP16, tag="y2t")
                    nc.scalar.activation(
                        y2t[:], y2p[:], mybir.ActivationFunctionType.Sigmoid,
                        bias=bo2_sb[:, 0:1], scale=1.0)

                    # --- outputs (feature-major) ---
                    pc0 = st["c0"]
                    nc.sync.dma_start(outT1[:, pc0:pc0 + MACRO], st["t"][0:O1, :])
                    nc.sync.dma_start(outT2[:, pc0:pc0 + MACRO], y2t[:])
    nc.compile()
    return nc


def _prep_inputs(X, W1, b1, Wo1, bo1, W2, b2, Wo2, bo2, rows_per_core, n_cores):
    """Host-side prep: shard + transpose X to fp16, split W1 hi/lo, pack."""
    X = np.asarray(X, dtype=np.float32)
    W1 = np.asarray(W1, dtype=np.float32)
    b1 = np.asarray(b1, dtype=np.float32)
    Wo1 = np.asarray(Wo1, dtype=np.float32)
    bo1 = np.asarray(bo1, dtype=np.float32)
    W2 = np.asarray(W2, dtype=np.float32)
    b2 = np.asarray(b2, dtype=np.float32)
    Wo2 = np.asarray(Wo2, dtype=np.float32)
    bo2 = np.asarray(bo2, dtype=np.float32)

    w1t = np.ascontiguousarray(W1.T)                     # [128, 256]
    w1t_hi = w1t.astype(np.float16)
    w1t_lo = (w1t - w1t_hi.astype(np.float32)).astype(np.float16)
    w2t = W2.T                                           # [138, 256]
    w2xt = np.ascontiguousarray(w2t[:IN]).astype(np.float16)
    # fold b2 into the one-hot weight columns: y1 @ (W2y + b2 1^T).T = y1@W2y.T + b2
    w2yt = np.ascontiguousarray(w2t[IN:] + b2[None, :]).astype(np.float16)
    wo1t = np.ascontiguousarray(Wo1.T)                   # [256, 10]
    wo1t_p = np.concatenate([wo1t[:128], wo1t[128:]], axis=1)  # [128, 20]
    wo2t = np.ascontiguousarray(Wo2.T).astype(np.float16)      # [256, 6]
    wo2t_p = np.concatenate([wo2t[:128], wo2t[128:]], axis=1)  # [128, 12]

    common = {
        "w1t_hi": w1t_hi, "w1t_lo": w1t_lo,
        "b1": np.ascontiguousarray(b1.reshape(2, 128).T),
        "wo1t": wo1t_p,
        "bo1n": np.ascontiguousarray(np.broadcast_to(bo1, (32, O1))),
        "w2xt": w2xt, "w2yt": w2yt,
        "wo2t": wo2t_p,
        "bo2": np.ascontiguousarray(bo2.reshape(O2, 1)),
    }

    in_maps = []
    for c in range(n_cores):
        Xs = X[c * rows_per_core:(c + 1) * rows_per_core]
        xt = np.ascontiguousarray(Xs.T).astype(np.float16)   # [128, rows]
        in_maps.append({**common, "xt": xt})
    return in_maps


_NC_CACHE = {}


def _get_nc(n_macros, bo1_nonzero):
    key = (n_macros, bo1_nonzero)
    if key not in _NC_CACHE:
        _NC_CACHE[key] = build(n_macros, bo1_nonzero)
    return _NC_CACHE[key]


def run(X, W1, b1, Wo1, bo1, W2, b2, Wo2, bo2, trace=False):
    """Full-size run across 8 cores. Returns (out [B,16] f32, exec_time_ns|None)."""
    n_macros = SHARD // MACRO
    bo1_nonzero = bool(np.any(np.asarray(bo1)))
    nc = _get_nc(n_macros, bo1_nonzero)
    in_maps = _prep_inputs(X, W1, b1, Wo1, bo1, W2, b2, Wo2, bo2, SHARD, N_CORES)
    res = run_bass_kernel_spmd(nc, in_maps, core_ids=list(range(N_CORES)), trace=trace)
    out = np.empty((BATCH, OUT), dtype=np.float32)
    for c in range(N_CORES):
        r = res.results[c]
        out[c * SHARD:(c + 1) * SHARD, :O1] = r["outT1"].T.astype(np.float32)
        out[c * SHARD:(c + 1) * SHARD, O1:] = r["outT2"].T.astype(np.float32)
    return out, res.exec_time_ns


def kernel(X, W1, b1, Wo1, bo1, W2, b2, Wo2, bo2):
    out, _ = run(X, W1, b1, Wo1, bo1, W2, b2, Wo2, bo2)
    return out


# revision 9
# speedup vs baseline: 1.2960x; 1.1321x over previous
"""Trainium2 Bass kernel for nn_AnimationPredictor (2-layer MLP with argmax/one-hot).

Data-parallel over 8 NeuronCores: each core processes 65536 rows.

Math per row (reference):
  h1 = relu(X @ W1.T + b1)            [B, 256]
  logits = h1 @ Wo1.T + bo1           [B, 10]
  y1 = one_hot(argmax(logits), 10)
  h2 = relu(concat([X, y1]) @ W2.T + b2)   [B, 256]
  y2 = sigmoid(h2 @ Wo2.T + bo2)      [B, 6]
  out = concat([y1, y2])              [B, 16]

On-chip layout: batch rows on the matmul free dim ("T layout"). X arrives
pre-transposed as fp16; stage 1 runs as Xh @ (W1hi + W1lo) with the weight
split hi/lo in fp16 (2-term: X-quantization only; ~1e-4 of rows flip
argmax, within the error budget). Logits are computed in T layout with
f32r operands (f32 exponent, 11-bit mantissa). The argmax machinery runs
at TWO-macro granularity (1024 rows) to amortize fixed per-op costs: a
32x32-block STREAM_TRANSPOSE turns the [32, 1024] logits psum into an
interleaved natural layout where the 10 classes sit on the free dim, so
max/one-hot are cheap free-dim ops; a second block transpose returns the
one-hot to T layout for stage 2. b2 is folded into W2y on the host
(one-hot rows sum to 1). y2 + sigmoid + the output DMAs also run at
two-macro granularity. bo1 is applied with a natural-layout add only when
nonzero (the reference uses zero biases). Outputs leave feature-major
fp16; the host transposes and casts.
"""
import sys

sys.path.insert(0, "/opt/trn_rl_repo")

import numpy as np

import concourse.bass as bass
import concourse.tile as tile
from concourse import bacc, mybir
from concourse.bass_utils import run_bass_kernel_spmd

F32 = mybir.dt.float32
F32R = mybir.dt.float32r
FP16 = mybir.dt.float16

N_CORES = 8
BATCH = 524288
IN = 128
H = 256
O1 = 10
O2 = 6
OUT = O1 + O2
SHARD = BATCH // N_CORES          # 65536 rows per core
MACRO = 512                       # rows per macro-tile (one PSUM bank at f32)
PAIR = 2 * MACRO                  # argmax/y2/DMA granularity


def build(n_macros=SHARD // MACRO, bo1_nonzero=False):
    assert n_macros % 2 == 0
    nc = bacc.Bacc("TRN2", target_bir_lowering=False, debug=False)
    rows = n_macros * MACRO

    # --- DRAM parameters (per-core shapes) ---
    xt = nc.dram_tensor("xt", [IN, rows], FP16, kind="ExternalInput").ap()
    w1t_hi = nc.dram_tensor("w1t_hi", [IN, H], FP16, kind="ExternalInput").ap()
    w1t_lo = nc.dram_tensor("w1t_lo", [IN, H], FP16, kind="ExternalInput").ap()
    b1_d = nc.dram_tensor("b1", [128, 2], F32, kind="ExternalInput").ap()
    wo1t_d = nc.dram_tensor("wo1t", [128, 2 * O1], F32R, kind="ExternalInput").ap()
    bo1n_d = nc.dram_tensor("bo1n", [32, O1], F32, kind="ExternalInput").ap()
    w2xt_d = nc.dram_tensor("w2xt", [IN, H], FP16, kind="ExternalInput").ap()
    w2yt_d = nc.dram_tensor("w2yt", [O1, H], FP16, kind="ExternalInput").ap()
    wo2t_d = nc.dram_tensor("wo2t", [128, 2 * O2], FP16, kind="ExternalInput").ap()
    bo2_d = nc.dram_tensor("bo2", [O2, 1], F32, kind="ExternalInput").ap()
    outT1 = nc.dram_tensor("outT1", [O1, rows], FP16, kind="ExternalOutput").ap()
    outT2 = nc.dram_tensor("outT2", [O2, rows], FP16, kind="ExternalOutput").ap()

    with tile.TileContext(nc) as tc:
        with tc.tile_pool(name="const", bufs=1) as cpool, \
             tc.tile_pool(name="xin", bufs=3) as xin, \
             tc.tile_pool(name="h1sb", bufs=6) as h1sb, \
             tc.tile_pool(name="nat", bufs=2) as nat, \
             tc.tile_pool(name="tt", bufs=4) as tt, \
             tc.tile_pool(name="h2sb", bufs=8) as h2sb, \
             tc.tile_pool(name="h1ps", bufs=2, space="PSUM") as h1ps, \
             tc.tile_pool(name="lgps", bufs=1, space="PSUM") as lgps, \
             tc.tile_pool(name="h2ps", bufs=2, space="PSUM") as h2ps, \
             tc.tile_pool(name="y2ps", bufs=1, space="PSUM") as y2ps:

            # --- constants into SBUF ---
            w1t_hi_sb = cpool.tile_from(w1t_hi)
            w1t_lo_sb = cpool.tile_from(w1t_lo)
            b1_sb = cpool.tile_from(b1_d)
            wo1t_sb = cpool.tile_from(wo1t_d)
            bo1n_sb = cpool.tile_from(bo1n_d)
            w2xt_sb = cpool.tile_from(w2xt_d)
            w2yt_sb = cpool.tile_from(w2yt_d)
            wo2t_sb = cpool.tile_from(wo2t_d)
            bo2_sb = cpool.tile_from(bo2_d)

            # Software pipeline over macros; argmax/y2/DMA work at pair
            # (2-macro) granularity. Stage order within an iteration: input
            # DMA + h1 (m), logits+argmax (pair ending at m-1), y2+sigmoid+
            # out-DMAs (pair ending at m-5), h2 (m-3). The PE never waits on
            # the DVE argmax chain or the relu/sigmoid ops.
            S = {}     # per-macro state
            P = {}     # per-pair state, keyed by pair index
            for m in range(n_macros + 6):
                if m < n_macros:
                    c0 = m * MACRO
                    if m % 2 == 0:
                        xp = xin.tile([IN, PAIR], FP16, tag="xp")
                        nc.sync.dma_start(xp[:], xt[:, c0:c0 + PAIR])
                    xh = xp[:, (m % 2) * MACRO:(m % 2 + 1) * MACRO]

                    # --- stage 1: h1T = relu((W1hi+W1lo) @ X.T + b1) ---
                    h1t = []
                    for c in range(2):
                        ps = h1ps.tile([128, MACRO], F32, tag="h1ps")
                        wh = w1t_hi_sb[:, 128 * c:128 * (c + 1)]
                        wl = w1t_lo_sb[:, 128 * c:128 * (c + 1)]
                        nc.tensor.matmul(ps[:], wh, xh, start=True, stop=False)
                        nc.tensor.matmul(ps[:], wl, xh, start=False, stop=True)
                        sb = h1sb.tile([128, MACRO], F32R, tag="h1")
                        if c == 0:
                            # relu on DVE: (psum + b1) max 0
                            nc.vector.tensor_scalar(
                                sb[:], ps[:], b1_sb[:, c:c + 1], 0.0,
                                mybir.AluOpType.add, mybir.AluOpType.max)
                        else:
                            # relu on ACT (engine balance)
                            nc.scalar.activation(
                                sb[:], ps[:], mybir.ActivationFunctionType.Relu,
                                bias=b1_sb[:, c:c + 1], scale=1.0)
                        h1t.append(sb)
                    S[m] = {"c0": c0, "xh": xh, "h1t": h1t}

                # --- logits + argmax for pair (m-2, m-1), at odd m-1 ---
                if m >= 2 and m % 2 == 0 and m - 2 in S:
                    pidx = (m - 2) // 2
                    lg = lgps.tile([32, PAIR], F32, tag="lg", name="lg")
                    for k in range(2):           # macro m-2+k
                        st = S[m - 2 + k]
                        for c in range(2):
                            nc.tensor.matmul(
                                lg[0:O1, MACRO * k:MACRO * (k + 1)],
                                wo1t_sb[:, O1 * c:O1 * (c + 1)],
                                st["h1t"][c][:],
                                start=(c == 0), stop=(c == 1))

                    # block-transpose: lgn[i, 32b + j] = logit_j(row 32b + i)
                    lgn = nat.tile([32, PAIR], F32, tag="lgn")
                    nc.vector.transpose(lgn[:], lg[:])
                    lgn3 = lgn[:].rearrange("p (b j) -> p b j", j=32)[:, :, 0:O1]
                    if bo1_nonzero:
                        bo1_b = bo1n_sb[:].unsqueeze(1).broadcast_to(
                            [32, PAIR // 32, O1])
                        nc.vector.tensor_tensor(
                            lgn3, lgn3, bo1_b, mybir.AluOpType.add)

                    # argmax -> one-hot (exact f32 compare)
                    mx = nat.tile([32, PAIR // 32], F32, tag="mx")
                    nc.vector.tensor_reduce(
                        out=mx[:], in_=lgn3, op=mybir.AluOpType.max,
                        axis=mybir.AxisListType.X)
                    oh = nat.tile([32, PAIR], FP16, tag="oh")
                    oh3 = oh[:].rearrange("p (b j) -> p b j", j=32)[:, :, 0:O1]
                    mx_b = mx[:].unsqueeze(2).broadcast_to([32, PAIR // 32, O1])
                    nc.vector.tensor_tensor(oh3, lgn3, mx_b, mybir.AluOpType.is_equal)

                    # block-transpose back: t[j, r] = one_hot, j in 0..9
                    t_out = tt.tile([32, PAIR], FP16, tag="t_out")
                    nc.vector.transpose(t_out[:], oh[:])
                    P[pidx] = {"t": t_out, "c0": S[m - 2]["c0"]}

                # --- y2 + sigmoid + output DMAs for pair (m-6, m-5) ---
                if m >= 6 and m % 2 == 0 and (m - 6) // 2 in P:
                    pidx = (m - 6) // 2
                    pst = P[pidx]
                    y2p = y2ps.tile([O2, PAIR], F32, tag="y2ps")
                    for k in range(2):           # macro m-6+k
                        h2t = S[m - 6 + k]["h2t"]
                        for c in range(2):
                            nc.tensor.matmul(
                                y2p[:, MACRO * k:MACRO * (k + 1)],
                                wo2t_sb[:, O2 * c:O2 * (c + 1)],
                                h2t[c][:], start=(c == 0), stop=(c == 1))
                    y2t = tt.tile([O2, PAIR], FP16, tag="y2t")
                    nc.scalar.activation(
                        y2t[:], y2p[:], mybir.ActivationFunctionType.Sigmoid,
                        bias=bo2_sb[:, 0:1], scale=1.0)

                    pc0 = pst["c0"]
                    nc.sync.dma_start(outT1[:, pc0:pc0 + PAIR], pst["t"][0:O1, :])
                    nc.sync.dma_start(outT2[:, pc0:pc0 + PAIR], y2t[:])
                    del P[pidx]
                    del S[m - 6]
                    del S[m - 5]

                # --- stage 2 for macro m-3: h2T = relu(W2x@X.T + W2y'@y1T) ---
                if m >= 3 and m - 3 in S:
                    st = S[m - 3]
                    y1t = P[(m - 3) // 2]["t"]
                    h2t = []
                    for c in range(2):
                        ps = h2ps.tile([128, MACRO], F32, tag="h2ps")
                        nc.tensor.matmul(
                            ps[:], w2xt_sb[:, 128 * c:128 * (c + 1)], st["xh"],
                            start=True, stop=False)
                        off = ((m - 3) % 2) * MACRO
                        nc.tensor.matmul(
                            ps[:], w2yt_sb[:, 128 * c:128 * (c + 1)],
                            y1t[0:O1, off:off + MACRO], start=False, stop=True)
                        sb = h2sb.tile([128, MACRO], FP16, tag="h2")
                        nc.scalar.activation(
                            sb[:], ps[:], mybir.ActivationFunctionType.Relu,
                            bias=0.0, scale=1.0)
                        h2t.append(sb)
                    st["h2t"] = h2t
    nc.compile()
    return nc


def _prep_inputs(X, W1, b1, Wo1, bo1, W2, b2, Wo2, bo2, rows_per_core, n_cores):
    """Host-side prep: shard + transpose X to fp16, split W1 hi/lo, pack."""
    X = np.asarray(X, dtype=np.float32)
    W1 = np.asarray(W1, dtype=np.float32)
    b1 = np.asarray(b1, dtype=np.float32)
    Wo1 = np.asarray(Wo1, dtype=np.float32)
    bo1 = np.asarray(bo1, dtype=np.float32)
    W2 = np.asarray(W2, dtype=np.float32)
    b2 = np.asarray(b2, dtype=np.float32)
    Wo2 = np.asarray(Wo2, dtype=np.float32)
    bo2 = np.asarray(bo2, dtype=np.float32)

    w1t = np.ascontiguousarray(W1.T)                     # [128, 256]
    w1t_hi = w1t.astype(np.float16)
    w1t_lo = (w1t - w1t_hi.astype(np.float32)).astype(np.float16)
    w2t = W2.T                                           # [138, 256]
    w2xt = np.ascontiguousarray(w2t[:IN]).astype(np.float16)
    # fold b2 into the one-hot weight columns: y1 @ (W2y + b2 1^T).T = y1@W2y.T + b2
    w2yt = np.ascontiguousarray(w2t[IN:] + b2[None, :]).astype(np.float16)
    wo1t = np.ascontiguousarray(Wo1.T)                   # [256, 10]
    wo1t_p = np.concatenate([wo1t[:128], wo1t[128:]], axis=1)  # [128, 20]
    wo2t = np.ascontiguousarray(Wo2.T).astype(np.float16)      # [256, 6]
    wo2t_p = np.concatenate([wo2t[:128], wo2t[128:]], axis=1)  # [128, 12]

    common = {
        "w1t_hi": w1t_hi, "w1t_lo": w1t_lo,
        "b1": np.ascontiguousarray(b1.reshape(2, 128).T),
        "wo1t": wo1t_p,
        "bo1n": np.ascontiguousarray(np.broadcast_to(bo1, (32, O1))),
        "w2xt": w2xt, "w2yt": w2yt,
        "wo2t": wo2t_p,
        "bo2": np.ascontiguousarray(bo2.reshape(O2, 1)),
    }

    in_maps = []
    for c in range(n_cores):
        Xs = X[c * rows_per_core:(c + 1) * rows_per_core]
        xt = np.ascontiguousarray(Xs.T).astype(np.float16)   # [128, rows]
        in_maps.append({**common, "xt": xt})
    return in_maps


_NC_CACHE = {}


def _get_nc(n_macros, bo1_nonzero):
    key = (n_macros, bo1_nonzero)
    if key not in _NC_CACHE:
        _NC_CACHE[key] = build(n_macros, bo1_nonzero)
    return _NC_CACHE[key]


def run(X, W1, b1, Wo1, bo1, W2, b2, Wo2, bo2, trace=False):
    """Full-size run across 8 cores. Returns (out [B,16] f32, exec_time_ns|None)."""
    n_macros = SHARD // MACRO
    bo1_nonzero = bool(np.any(np.asarray(bo1)))
    nc = _get_nc(n_macros, bo1_nonzero)
    in_maps = _prep_inputs(X, W1, b1, Wo1, bo1, W2, b2, Wo2, bo2, SHARD, N_CORES)
    res = run_bass_kernel_spmd(nc, in_maps, core_ids=list(range(N_CORES)), trace=trace)
    out = np.empty((BATCH, OUT), dtype=np.float32)
    for c in range(N_CORES):
        r = res.results[c]
        out[c * SHARD:(c + 1) * SHARD, :O1] = r["outT1"].T.astype(np.float32)
        out[c * SHARD:(c + 1) * SHARD, O1:] = r["outT2"].T.astype(np.float32)
    return out, res.exec_time_ns


def kernel(X, W1, b1, Wo1, bo1, W2, b2, Wo2, bo2):
    out, _ = run(X, W1, b1, Wo1, bo1, W2, b2, Wo2, bo2)
    return out


# revision 11
# speedup vs baseline: 1.3958x; 1.0770x over previous
"""Trainium2 Bass kernel for nn_AnimationPredictor (2-layer MLP with argmax/one-hot).

Data-parallel over 8 NeuronCores: each core processes 65536 rows.

Math per row (reference):
  h1 = relu(X @ W1.T + b1)            [B, 256]
  logits = h1 @ Wo1.T + bo1           [B, 10]
  y1 = one_hot(argmax(logits), 10)
  h2 = relu(concat([X, y1]) @ W2.T + b2)   [B, 256]
  y2 = sigmoid(h2 @ Wo2.T + bo2)      [B, 6]
  out = concat([y1, y2])              [B, 16]

On-chip layout: batch rows on the matmul free dim ("T layout"). X arrives
pre-transposed as fp16; stage 1 runs as Xh @ (W1hi + W1lo) with the weight
split hi/lo in fp16 (2-term: X-quantization only; ~1e-4 of rows flip
argmax, within the error budget). Logits are computed in T layout with
f32r operands (f32 exponent, 11-bit mantissa). The argmax machinery runs
at TWO-macro granularity (1024 rows) to amortize fixed per-op costs: a
32x32-block STREAM_TRANSPOSE turns the [32, 1024] logits psum into an
interleaved natural layout where the 10 classes sit on the free dim, so
max/one-hot are cheap free-dim ops; a second block transpose returns the
one-hot to T layout for stage 2. b2 is folded into W2y on the host
(one-hot rows sum to 1). y2 + sigmoid + the output DMAs also run at
two-macro granularity. bo1 is applied with a natural-layout add only when
nonzero (the reference uses zero biases). Outputs leave feature-major
fp16; the host transposes and casts.
"""
import sys

sys.path.insert(0, "/opt/trn_rl_repo")

import numpy as np

import concourse.bass as bass
import concourse.tile as tile
from concourse import bacc, mybir
from concourse.bass_utils import run_bass_kernel_spmd

F32 = mybir.dt.float32
F32R = mybir.dt.float32r
FP16 = mybir.dt.float16

N_CORES = 8
BATCH = 524288
IN = 128
H = 256
O1 = 10
O2 = 6
OUT = O1 + O2
SHARD = BATCH // N_CORES          # 65536 rows per core
MACRO = 512                       # rows per macro-tile (one PSUM bank at f32)
PAIR = 2 * MACRO                  # argmax/y2/DMA granularity
# fp16 logits path: fold x256 into W1/b1 (relu commutes with positive
# scale; h1 only feeds the logits) and x16 into Wo1, so the logits matmul
# runs in plain fp16 (1 cyc/row, small LDW) with no subnormal loss.
# Costs ~40 extra argmax flips vs f32r (sim: rel 1.69e-2 vs 1.47e-2).
LG_FP16 = False
S1 = 256.0
SO = 16.0


def build(n_macros=SHARD // MACRO, bo1_nonzero=False):
    assert n_macros % 2 == 0
    nc = bacc.Bacc("TRN2", target_bir_lowering=False, debug=False)
    rows = n_macros * MACRO

    # --- DRAM parameters (per-core shapes) ---
    xt = nc.dram_tensor("xt", [IN, rows], FP16, kind="ExternalInput").ap()
    w1t_hi = nc.dram_tensor("w1t_hi", [IN, H], FP16, kind="ExternalInput").ap()
    w1t_lo = nc.dram_tensor("w1t_lo", [IN, H], FP16, kind="ExternalInput").ap()
    b1_d = nc.dram_tensor("b1", [128, 2], F32, kind="ExternalInput").ap()
    wo1t_d = nc.dram_tensor("wo1t", [128, 2 * O1],
                            FP16 if LG_FP16 else F32R,
                            kind="ExternalInput").ap()
    bo1n_d = nc.dram_tensor("bo1n", [32, O1], F32, kind="ExternalInput").ap()
    w2xt_d = nc.dram_tensor("w2xt", [IN, H], FP16, kind="ExternalInput").ap()
    w2yt_d = nc.dram_tensor("w2yt", [O1, H], FP16, kind="ExternalInput").ap()
    wo2t_d = nc.dram_tensor("wo2t", [128, 2 * O2], FP16, kind="ExternalInput").ap()
    bo2_d = nc.dram_tensor("bo2", [O2, 1], F32, kind="ExternalInput").ap()
    outT1 = nc.dram_tensor("outT1", [O1, rows], FP16, kind="ExternalOutput").ap()
    outT2 = nc.dram_tensor("outT2", [O2, rows], FP16, kind="ExternalOutput").ap()

    with tile.TileContext(nc) as tc:
        with tc.tile_pool(name="const", bufs=1) as cpool, \
             tc.tile_pool(name="xin", bufs=3) as xin, \
             tc.tile_pool(name="h1sb", bufs=6) as h1sb, \
             tc.tile_pool(name="nat", bufs=2) as nat, \
             tc.tile_pool(name="tt", bufs=4) as tt, \
             tc.tile_pool(name="h2sb", bufs=8) as h2sb, \
             tc.tile_pool(name="h1ps", bufs=2, space="PSUM") as h1ps, \
             tc.tile_pool(name="lgps", bufs=1, space="PSUM") as lgps, \
             tc.tile_pool(name="h2ps", bufs=2, space="PSUM") as h2ps, \
             tc.tile_pool(name="y2ps", bufs=1, space="PSUM") as y2ps:

            # --- constants into SBUF ---
            w1t_hi_sb = cpool.tile_from(w1t_hi)
            w1t_lo_sb = cpool.tile_from(w1t_lo)
            b1_sb = cpool.tile_from(b1_d)
            wo1t_sb = cpool.tile_from(wo1t_d)
            bo1n_sb = cpool.tile_from(bo1n_d)
            w2xt_sb = cpool.tile_from(w2xt_d)
            w2yt_sb = cpool.tile_from(w2yt_d)
            wo2t_sb = cpool.tile_from(wo2t_d)
            bo2_sb = cpool.tile_from(bo2_d)

            # Software pipeline over macros; argmax/y2/DMA work at pair
            # (2-macro) granularity. Stage order within an iteration: input
            # DMA + h1 (m), logits+argmax (pair ending at m-1), y2+sigmoid+
            # out-DMAs (pair ending at m-5), h2 (m-3). The PE never waits on
            # the DVE argmax chain or the relu/sigmoid ops.
            S = {}     # per-macro state
            P = {}     # per-pair state, keyed by pair index
            for m in range(n_macros + 7):
                if m < n_macros:
                    c0 = m * MACRO
                    if m % 2 == 0:
                        xp = xin.tile([IN, PAIR], FP16, tag="xp")
                        nc.sync.dma_start(xp[:], xt[:, c0:c0 + PAIR])
                    xh = xp[:, (m % 2) * MACRO:(m % 2 + 1) * MACRO]

                    # --- stage 1: h1T = relu((W1hi+W1lo) @ X.T + b1) ---
                    h1t = []
                    for c in range(2):
                        ps = h1ps.tile([128, MACRO], F32, tag="h1ps")
                        wh = w1t_hi_sb[:, 128 * c:128 * (c + 1)]
                        wl = w1t_lo_sb[:, 128 * c:128 * (c + 1)]
                        nc.tensor.matmul(ps[:], wh, xh, start=True, stop=False)
                        nc.tensor.matmul(ps[:], wl, xh, start=False, stop=True)
                        sb = h1sb.tile([128, MACRO],
                                       FP16 if LG_FP16 else F32R, tag="h1")
                        if c == 0:
                            # relu on DVE: (psum + b1) max 0
                            nc.vector.tensor_scalar(
                                sb[:], ps[:], b1_sb[:, c:c + 1], 0.0,
                                mybir.AluOpType.add, mybir.AluOpType.max)
                        else:
                            # relu on ACT (engine balance)
                            nc.scalar.activation(
                                sb[:], ps[:], mybir.ActivationFunctionType.Relu,
                                bias=b1_sb[:, c:c + 1], scale=1.0)
                        h1t.append(sb)
                    S[m] = {"c0": c0, "xh": xh, "h1t": h1t}

                # --- logits + argmax for pair (m-2, m-1), at odd m-1 ---
                if m >= 2 and m % 2 == 0 and m - 2 in S:
                    pidx = (m - 2) // 2
                    lg = lgps.tile([32, PAIR], F32, tag="lg", name="lg")
                    for k in range(2):           # macro m-2+k
                        st = S[m - 2 + k]
                        for c in range(2):
                            nc.tensor.matmul(
                                lg[0:O1, MACRO * k:MACRO * (k + 1)],
                                wo1t_sb[:, O1 * c:O1 * (c + 1)],
                                st["h1t"][c][:],
                                start=(c == 0), stop=(c == 1))

                    # block-transpose: lgn[i, 32b + j] = logit_j(row 32b + i)
                    lgn = nat.tile([32, PAIR], F32, tag="lgn")
                    nc.vector.transpose(lgn[:], lg[:])
                    lgn3 = lgn[:].rearrange("p (b j) -> p b j", j=32)[:, :, 0:O1]
                    if bo1_nonzero:
                        bo1_b = bo1n_sb[:].unsqueeze(1).broadcast_to(
                            [32, PAIR // 32, O1])
                        nc.vector.tensor_tensor(
                            lgn3, lgn3, bo1_b, mybir.AluOpType.add)

                    # argmax -> one-hot (exact f32 compare)
                    mx = nat.tile([32, PAIR // 32], F32, tag="mx")
                    nc.vector.tensor_reduce(
                        out=mx[:], in_=lgn3, op=mybir.AluOpType.max,
                        axis=mybir.AxisListType.X)
                    oh = nat.tile([32, PAIR], FP16, tag="oh")
                    oh3 = oh[:].rearrange("p (b j) -> p b j", j=32)[:, :, 0:O1]
                    mx_b = mx[:].unsqueeze(2).broadcast_to([32, PAIR // 32, O1])
                    nc.vector.tensor_tensor(oh3, lgn3, mx_b, mybir.AluOpType.is_equal)

                    # block-transpose back: t[j, r] = one_hot, j in 0..9
                    t_out = tt.tile([32, PAIR], FP16, tag="t_out")
                    nc.vector.transpose(t_out[:], oh[:])
                    P[pidx] = {"t": t_out, "c0": S[m - 2]["c0"]}

                # --- y2 + sigmoid + output DMAs for pair (m-7, m-6) ---
                if m >= 7 and m % 2 == 1 and (m - 7) // 2 in P:
                    pidx = (m - 7) // 2
                    pst = P[pidx]
                    y2p = y2ps.tile([O2, PAIR], F32, tag="y2ps")
                    for k in range(2):           # macro m-7+k
                        h2t = S[m - 7 + k]["h2t"]
                        for c in range(2):
                            nc.tensor.matmul(
                                y2p[:, MACRO * k:MACRO * (k + 1)],
                                wo2t_sb[:, O2 * c:O2 * (c + 1)],
                                h2t[c][:], start=(c == 0), stop=(c == 1))
                    y2t = tt.tile([O2, PAIR], FP16, tag="y2t")
                    nc.scalar.activation(
                        y2t[:], y2p[:], mybir.ActivationFunctionType.Sigmoid,
                        bias=bo2_sb[:, 0:1], scale=1.0)

                    pc0 = pst["c0"]
                    nc.sync.dma_start(outT1[:, pc0:pc0 + PAIR], pst["t"][0:O1, :])
                    nc.sync.dma_start(outT2[:, pc0:pc0 + PAIR], y2t[:])
                    del P[pidx]
                    del S[m - 7]
                    del S[m - 6]

                # --- stage 2 for macro m-3: h2T = relu(W2x@X.T + W2y'@y1T) ---
                if m >= 3 and m - 3 in S:
                    st = S[m - 3]
                    y1t = P[(m - 3) // 2]["t"]
                    off = ((m - 3) % 2) * MACRO
                    pss = []
                    for c in range(2):
                        ps = h2ps.tile([128, MACRO], F32, tag="h2ps")
                        nc.tensor.matmul(
                            ps[:], w2xt_sb[:, 128 * c:128 * (c + 1)], st["xh"],
                            start=True, stop=False)
                        pss.append(ps)
                    h2t = []
                    for c in range(2):
                        ps = pss[c]
                        nc.tensor.matmul(
                            ps[:], w2yt_sb[:, 128 * c:128 * (c + 1)],
                            y1t[0:O1, off:off + MACRO], start=False, stop=True)
                        sb = h2sb.tile([128, MACRO], FP16, tag="h2")
                        nc.scalar.activation(
                            sb[:], ps[:], mybir.ActivationFunctionType.Relu,
                            bias=0.0, scale=1.0)
                        h2t.append(sb)
                    st["h2t"] = h2t
    nc.compile()
    return nc


def _prep_inputs(X, W1, b1, Wo1, bo1, W2, b2, Wo2, bo2, rows_per_core, n_cores):
    """Host-side prep: shard + transpose X to fp16, split W1 hi/lo, pack."""
    X = np.asarray(X, dtype=np.float32)
    W1 = np.asarray(W1, dtype=np.float32)
    b1 = np.asarray(b1, dtype=np.float32)
    Wo1 = np.asarray(Wo1, dtype=np.float32)
    bo1 = np.asarray(bo1, dtype=np.float32)
    W2 = np.asarray(W2, dtype=np.float32)
    b2 = np.asarray(b2, dtype=np.float32)
    Wo2 = np.asarray(Wo2, dtype=np.float32)
    bo2 = np.asarray(bo2, dtype=np.float32)

    s1 = S1 if LG_FP16 else 1.0
    so = SO if LG_FP16 else 1.0
    w1t = np.ascontiguousarray(W1.T) * s1                # [128, 256]
    w1t_hi = w1t.astype(np.float16)
    w1t_lo = (w1t - w1t_hi.astype(np.float32)).astype(np.float16)
    w2t = W2.T                                           # [138, 256]
    w2xt = np.ascontiguousarray(w2t[:IN]).astype(np.float16)
    # fold b2 into the one-hot weight columns: y1 @ (W2y + b2 1^T).T = y1@W2y.T + b2
    w2yt = np.ascontiguousarray(w2t[IN:] + b2[None, :]).astype(np.float16)
    wo1t = np.ascontiguousarray(Wo1.T) * so              # [256, 10]
    wo1t_p = np.concatenate([wo1t[:128], wo1t[128:]], axis=1)  # [128, 20]
    if LG_FP16:
        wo1t_p = wo1t_p.astype(np.float16)
    wo2t = np.ascontiguousarray(Wo2.T).astype(np.float16)      # [256, 6]
    wo2t_p = np.concatenate([wo2t[:128], wo2t[128:]], axis=1)  # [128, 12]

    common = {
        "w1t_hi": w1t_hi, "w1t_lo": w1t_lo,
        "b1": np.ascontiguousarray((b1 * s1).reshape(2, 128).T),
        "wo1t": wo1t_p,
        "bo1n": np.ascontiguousarray(np.broadcast_to(bo1 * (s1 * so), (32, O1))),
        "w2xt": w2xt, "w2yt": w2yt,
        "wo2t": wo2t_p,
        "bo2": np.ascontiguousarray(bo2.reshape(O2, 1)),
    }

    in_maps = []
    for c in range(n_cores):
        Xs = X[c * rows_per_core:(c + 1) * rows_per_core]
        xt = np.ascontiguousarray(Xs.T).astype(np.float16)   # [128, rows]
        in_maps.append({**common, "xt": xt})
    return in_maps


_NC_CACHE = {}


def _get_nc(n_macros, bo1_nonzero):
    key = (n_macros, bo1_nonzero)
    if key not in _NC_CACHE:
        _NC_CACHE[key] = build(n_macros, bo1_nonzero)
    return _NC_CACHE[key]


def run(X, W1, b1, Wo1, bo1, W2, b2, Wo2, bo2, trace=False):
    """Full-size run across 8 cores. Returns (out [B,16] f32, exec_time_ns|None)."""
    n_macros = SHARD // MACRO
    bo1_nonzero = bool(np.any(np.asarray(bo1)))
    nc = _get_nc(n_macros, bo1_nonzero)
    in_maps = _prep_inputs(X, W1, b1, Wo1, bo1, W2, b2, Wo2, bo2, SHARD, N_CORES)
    res = run_bass_kernel_spmd(nc, in_maps, core_ids=list(range(N_CORES)), trace=trace)
    out = np.empty((BATCH, OUT), dtype=np.float32)
    for c in range(N_CORES):
        r = res.results[c]
        out[c * SHARD:(c + 1) * SHARD, :O1] = r["outT1"].T.astype(np.float32)
        out[c * SHARD:(c + 1) * SHARD, O1:] = r["outT2"].T.astype(np.float32)
    return out, res.exec_time_ns


def kernel(X, W1, b1, Wo1, bo1, W2, b2, Wo2, bo2):
    out, _ = run(X, W1, b1, Wo1, bo1, W2, b2, Wo2, bo2)
    return out
